# revision 19
# baseline (speedup 1.0000x reference)
"""GRU decoder (nn_Decoder2) Trainium2 Bass kernel, v2.

Per core (pure batch data-parallel over 8 cores): 4096 rows, 8 chunks of 512,
2 supergroups (SG) of 4 chunks, partition-stacked [128, 512] so elementwise
runs 128 lanes wide.  HW: ~289us (baseline 572us), rel err 6.2e-4.

Structure (vs v1's 4 narrow matmuls per gate-bank per SG):
  - Each gate bank [128, 512] (4 chunks stacked at partition 32*ci) is
    computed by TWO K-packed matmuls:
      feat-mm: lhsT [65, 128] block-diag feat weights (rows 16ci+f), row 64
               = ones/bias row; rhs = packed feat [65, 512]; start=True.
      h-mm:    lhsT [128, 128] block-diag recurrent weights; rhs = the
               stacked state tile h_s [128, 512] read DIRECTLY; no
               SBUF->SBUF state-scatter DMAs exist at all.
    mh has no x-side term (bias applied inside the t2 scalar_tensor_tensor);
    7 gate matmuls per SG per step vs 16 in v1.
  - t3 = t2 + xh is done ON THE PE: an identity-matmul accumulates t2 into
    the xh psum bank (closing its group); tanh then reads psum directly.
    This keeps DVE to 4 ops/SG: stt t2, d = h-hh, e = z*d, h' = hh+e
    (the last three all-fp16 SBUF -> DVE 2x mode, ~420ns each).
  - Sigmoid split R-then-Z [128, 512] each (r reaches the t2 chain ~0.5us
    earlier than a fused [128, 1024] sigmoid).
  - h_s is 5x ring-buffered: blend(t) writes buf[t%5], h-mms(t) read
    buf[(t+4)%5], so the dense head can read states up to 4 steps late.
  - Dense head batched per 4 steps: 8 mms (wd4 block-diag, one per step x
    SG) park at partition offsets 32*tau of the freed Z psum region, one
    DVE tensor_copy evacuates all 4 parks, 4 out-DMAs per SG per block.
  - feat prefetched per SG in 4-step blocks [70, 2048] fp16 (rows 65:69 =
    per-chunk (y0 - dense_b) at t=0, zeros later), multi-buffered.
  - Emission order IS the dependency order for the Tile framework: feats
    for step t+1 are emitted after step t's elementwise so the start=True
    bank wipes are WAR-ordered behind this step's psum reads.
Dense(1) -> next-input dependency folded into the recurrent weights
host-side (rk + dw x k0, bias += db*k0), so the recurrence never waits on
the dense output; y0 enters via feat rows 65:69 at t=0 (the y0-db trick
makes one weight set serve all t); host adds dense_b at the end.  Weights
are prepared in float64 and quantized to fp16 (PSUM accumulates fp32).
"""
import numpy as np

B, T, F, H = 32768, 48, 16, 32
NCORES = 8
BS = B // NCORES            # 4096 batch per core
CK = 512                    # chunk batch size
NSG = 2                     # supergroups
SGC = 4                     # chunks per supergroup

_CACHE = {}


def _prep_weights(kernel, recurrent_kernel, bias_x, bias_h, dense_w, dense_b):
    """Build v2 weight tiles in float64, return fp32 dict."""
    kd = kernel.astype(np.float64)
    rkd = recurrent_kernel.astype(np.float64)
    bxd = bias_x.astype(np.float64)
    bhd = bias_h.astype(np.float64)
    dwd = dense_w.astype(np.float64)[:, 0]          # [32]
    dbd = float(dense_b.astype(np.float64)[0])

    k0 = kd[0]                                      # [96]
    kf = kd[1:]                                     # [16, 96]
    dwk0 = np.outer(dwd, k0)                        # [32, 96]

    out = {}
    blocks = {"z": slice(0, 32), "r": slice(32, 64), "x": slice(64, 96)}
    bias1 = {"z": bxd[0:32] + bhd[0:32], "r": bxd[32:64] + bhd[32:64],
             "x": bxd[64:96]}
    for g, blk in blocks.items():
        # Unified feat weights, all t (t=0 handled by y0-db rhs rows):
        # row 0:64 = block-diag kf, 64 = bias1 + db*k0, 65:69 = k0 per
        # chunk, 69 = pad. DoubleRow-packed [35, 2, 128] -> [35, 256].
        wfr = np.zeros((70, 128), np.float64)
        whh = np.zeros((128, 128), np.float64)
        for c in range(4):
            cols = slice(32 * c, 32 * c + 32)
            wfr[16 * c:16 * c + 16, cols] = kf[:, blk]
            wfr[64, cols] = bias1[g] + dbd * k0[blk]
            wfr[65 + c, cols] = k0[blk]
            rows = slice(32 * c, 32 * c + 32)
            if g == "x":
                whh[rows, cols] = dwk0[:, blk]
            else:
                whh[rows, cols] = rkd[:, blk] + dwk0[:, blk]
        out[f"wf_{g}"] = wfr                    # [70, 128] unified, all t
        out[f"whh_{g}"] = whh
        if g != "x":
            wh0 = np.zeros((128, 128), np.float64)
            for c in range(4):
                wh0[32 * c:32 * c + 32, 32 * c:32 * c + 32] = rkd[:, blk]
            out[f"wh0_{g}"] = wh0
    whm = np.zeros((128, 128), np.float64)          # mh: h-only, all t
    for c in range(4):
        whm[32 * c:32 * c + 32, 32 * c:32 * c + 32] = rkd[:, 64:96]
    out["whh_m"] = whm
    wd4 = np.zeros((128, 4), np.float64)
    for c in range(4):
        wd4[32 * c:32 * c + 32, c] = dwd
    out["wd4"] = wd4
    out["bhm"] = np.tile(bhd[64:96], 4).reshape(128, 1)
    out["ident"] = np.eye(128)
    return {k: np.ascontiguousarray(v.astype(np.float32)) for k, v in out.items()}


def _build_module(n_steps=T):
    import concourse.bacc as bacc
    import concourse.mybir as mybir
    import concourse.tile as tile
    from contextlib import ExitStack

    f32 = mybir.dt.float32
    f16 = mybir.dt.float16
    AF = mybir.ActivationFunctionType
    ALU = mybir.AluOpType

    nc = bacc.Bacc("TRN2")
    # feat_packed [T, 70, 1024] fp16: rows 0:64 = chunk-stacked feats,
    # row 64 = ones, rows 65:69 = per-chunk (y0-db) at t=0 / zeros later.
    feat = nc.dram_tensor("feat", [n_steps, 70, NSG * CK], f16,
                          kind="ExternalInput")
    h0 = nc.dram_tensor("h0", [H, BS], f16, kind="ExternalInput")
    wnames_f = ["wf_z", "wf_r", "wf_x"]
    wnames_h = ["whh_z", "whh_r", "whh_x", "whh_m"]
    wnames_h0 = ["wh0_z", "wh0_r", "whh_m"]
    dram_w = {}
    for n in wnames_f:
        dram_w[n] = nc.dram_tensor(n, [70, 128], f16, kind="ExternalInput")
    for n in set(wnames_h + wnames_h0):
        dram_w[n] = nc.dram_tensor(n, [128, 128], f16, kind="ExternalInput")
    dram_w["wd4"] = nc.dram_tensor("wd4", [128, 4], f16, kind="ExternalInput")
    dram_w["ident"] = nc.dram_tensor("ident", [128, 128], f16,
                                     kind="ExternalInput")
    dram_w["bhm"] = nc.dram_tensor("bhm", [128, 1], f32, kind="ExternalInput")
    out = nc.dram_tensor("out", [n_steps, BS], f32, kind="ExternalOutput")

    NBLK = (n_steps + 3) // 4

    with tile.TileContext(nc) as tc, ExitStack() as ctx:
        wpool = ctx.enter_context(tc.tile_pool(name="weights", bufs=1))
        xpool = ctx.enter_context(tc.tile_pool(name="featp", bufs=3))
        hpool = ctx.enter_context(tc.tile_pool(name="hs", bufs=1))
        ew = ctx.enter_context(tc.tile_pool(name="ew", bufs=3))
        dpool = ctx.enter_context(tc.tile_pool(name="dsb", bufs=2))
        ppool = ctx.enter_context(tc.tile_pool(name="psum", bufs=1,
                                               space="PSUM"))

        ws = {}
        for n, d in dram_w.items():
            dt = f32 if n == "bhm" else f16
            t_ = wpool.tile(list(d.shape), dt, tag=n)
            nc.sync.dma_start(t_[:, :], d[:, :])
            ws[n] = t_

        # 5x-buffered stacked state tiles per SG [128, 512] fp16:
        # blend(t) writes buf[t%5]; h-mms(t) read buf[(t+4)%5]; the dense
        # head reads buf[tau%5] up to 5 steps later. h0 preloaded to buf 4.
        h_s = [[hpool.tile([128, CK], f16, tag=f"hs{g}_{p}", name=f"hs{g}_{p}")
                for p in range(5)] for g in range(NSG)]
        for g in range(NSG):
            for ci in range(SGC):
                c = g * SGC + ci
                nc.sync.dma_start(h_s[g][4][32 * ci:32 * ci + 32, :],
                                  h0[:, c * CK:(c + 1) * CK])

        # Feat block tiles [70, 2048] per SG (block b = steps 4b..4b+3).
        feat_t = [[None] * NBLK for _ in range(NSG)]

        def load_block(b):
            t0 = 4 * b
            ns = min(t0 + 4, n_steps) - t0
            for g in range(NSG):
                t_ = xpool.tile([70, 4 * CK], f16, tag=f"feat{g}",
                                name=f"feat{g}_b{b}")
                nc.sync.dma_start(
                    t_[0:70, 0:ns * CK].rearrange("r (s b) -> r s b", s=ns),
                    feat[t0:t0 + ns, :, g * CK:(g + 1) * CK].rearrange(
                        "s r b -> r s b"))
                feat_t[g][b] = t_

        load_block(0)
        if NBLK > 1:
            load_block(1)

        zr_ps = [ppool.tile([128, 2 * CK], f32, tag=f"zr{g}", name=f"zr{g}")
                 for g in range(NSG)]
        xm_ps = [ppool.tile([128, 2 * CK], f32, tag=f"xm{g}", name=f"xm{g}")
                 for g in range(NSG)]

        def gate_bank(g, suf):
            if suf == "_r":
                return zr_ps[g][:, CK:2 * CK]
            if suf == "_z":
                return zr_ps[g][:, 0:CK]
            if suf == "_m":
                return xm_ps[g][:, CK:2 * CK]
            return xm_ps[g][:, 0:CK]       # _x

        def emit_feat(tt, sufs=("_r", "_z", "_x")):
            bb, ss = tt // 4, tt % 4
            kk = 69 if tt == 0 else 65
            for suf in sufs:
                wn = [n for n in wnames_f if n.endswith(suf)][0]
                for g in range(NSG):
                    rhs = feat_t[g][bb][0:kk, ss * CK:(ss + 1) * CK]
                    nc.tensor.matmul(gate_bank(g, suf),
                                     lhsT=ws[wn][0:kk, :], rhs=rhs,
                                     start=True, stop=False,
                                     tile_position=(0, 0))

        def emit_dense_block(t0, nt):
            """Dense mms for steps t0..t0+nt-1, parked at partition offset
            32*tau of the Z psum region, one ACT evac, per-step out-DMAs.
            Emitted during step t0+4 (or after the loop)."""
            for g in range(NSG):
                for tau in range(nt):
                    p0 = 32 * tau
                    nc.tensor.matmul(zr_ps[g][p0:p0 + 4, 0:CK],
                                     lhsT=ws["wd4"][:, :],
                                     rhs=h_s[g][(t0 + tau) % 5][:, :],
                                     start=True, stop=True,
                                     tile_position=(0, p0))
            for g in range(NSG):
                npp = 32 * (nt - 1) + 4
                dsb = dpool.tile([100, CK], f32, tag=f"dsb{g}",
                                 name=f"dsb{g}_{t0}")
                nc.vector.tensor_copy(dsb[0:npp, :], zr_ps[g][0:npp, 0:CK])
                gb = g * SGC * CK
                for tau in range(nt):
                    nc.sync.dma_start(
                        out[t0 + tau, gb:gb + 4 * CK].rearrange(
                            "(c b) -> c b", c=4),
                        dsb[32 * tau:32 * tau + 4, :])

        for t in range(n_steps):
            blk, slot = t // 4, t % 4
            wh = wnames_h0 if t == 0 else wnames_h  # feat weights unified
            hb_in = [h_s[g][(t + 4) % 5] for g in range(NSG)]

            if t == 0:
                emit_feat(0)

            # -- PE: h-mms, gate-paired across SGs; r first ------------
            for suf in ["_r", "_z", "_m", "_x"]:
                wn = [n for n in wh if n.endswith(suf)]
                if not wn:
                    continue
                for g in range(NSG):
                    # xh bank is closed by the t2-inject mm, not here
                    nc.tensor.matmul(gate_bank(g, suf), lhsT=ws[wn[0]][:, :],
                                     rhs=hb_in[g][:, :],
                                     start=(suf == "_m"),
                                     stop=(suf != "_x"),
                                     tile_position=(0, 0))

            # -- elementwise per SG, op-major emission ----------------
            ewt = {}
            for g in range(NSG):
                ewt[g] = (
                    ew.tile([128, 2 * CK], f16, tag=f"zrs{g}",
                            name=f"zrs{g}_{t}"),
                    ew.tile([128, CK], f16, tag=f"t2s{g}", name=f"t2s{g}_{t}"),
                    None,
                    ew.tile([128, CK], f16, tag=f"hhs{g}", name=f"hhs{g}_{t}"),
                    ew.tile([128, CK], f16, tag=f"ds{g}", name=f"ds{g}_{t}"),
                    ew.tile([128, CK], f16, tag=f"es{g}", name=f"es{g}_{t}"),
                )
            for g in range(NSG):
                nc.scalar.activation(ewt[g][0][:, CK:2 * CK],
                                     zr_ps[g][:, CK:2 * CK], AF.Sigmoid)
            for g in range(NSG):
                nc.vector.scalar_tensor_tensor(
                    ewt[g][1][:, :], xm_ps[g][:, CK:2 * CK], ws["bhm"][:, 0:1],
                    ewt[g][0][:, CK:2 * CK], ALU.add, ALU.mult)
            for g in range(NSG):
                # t3: accumulate t2 into the xh psum bank on PE (identity
                # matmul, closes the bank group); tanh then reads psum.
                nc.tensor.matmul(xm_ps[g][:, 0:CK], lhsT=ws["ident"][:, :],
                                 rhs=ewt[g][1][:, :], start=False, stop=True,
                                 tile_position=(0, 0))
            # feat-r(t+1) right after its WAR reader sigR(t): the PE can
            # reopen the r bank as soon as sigR completes, well before the
            # end of this step's elementwise.
            dense_step = t > 0 and t % 4 == 0
            if t + 1 < n_steps:
                emit_feat(t + 1, ("_r",))
            for g in range(NSG):
                nc.scalar.activation(ewt[g][0][:, 0:CK],
                                     zr_ps[g][:, 0:CK], AF.Sigmoid)
            if t + 1 < n_steps and not dense_step:
                # feat-z(t+1) after sigZ(t); deferred past the dense block
                # on park steps (the parks write the Z region).
                emit_feat(t + 1, ("_z",))
            for g in range(NSG):
                nc.scalar.activation(ewt[g][3][:, :], xm_ps[g][:, 0:CK],
                                     AF.Tanh)
            if t + 1 < n_steps:
                emit_feat(t + 1, ("_x",))
            for g in range(NSG):
                nc.vector.tensor_sub(ewt[g][4][:, :], hb_in[g][:, :],
                                     ewt[g][3][:, :])
            for g in range(NSG):
                nc.vector.tensor_mul(ewt[g][5][:, :], ewt[g][0][:, 0:CK],
                                     ewt[g][4][:, :])
            for g in range(NSG):
                nc.vector.tensor_add(h_s[g][t % 5][:, :], ewt[g][3][:, :],
                                     ewt[g][5][:, :])

            # -- PE: dense block for previous 4 steps (after sig(t) so the
            #    parked writes are WAR-ordered behind the Z-region read) --
            if dense_step:
                emit_dense_block(t - 4, 4)
                if t + 1 < n_steps:
                    emit_feat(t + 1, ("_z",))

            # -- prefetch feat block -----------------------------------
            if slot == 3 and blk + 2 < NBLK:
                load_block(blk + 2)

        last0 = (n_steps - 1) // 4 * 4
        emit_dense_block(last0, n_steps - last0)
    nc.compile()
    return nc


def _host_prep(inputs, n_steps=T):
    """Shard + pack inputs host-side. Returns (in_maps, dense_b)."""
    dfeat = np.asarray(inputs["decoder_feature"], np.float32)
    y0 = np.asarray(inputs["decoder_init_input"], np.float32)
    h0 = np.asarray(inputs["init_state"], np.float32)
    ws = _prep_weights(
        np.asarray(inputs["kernel"], np.float32),
        np.asarray(inputs["recurrent_kernel"], np.float32),
        np.asarray(inputs["bias_x"], np.float32),
        np.asarray(inputs["bias_h"], np.float32),
        np.asarray(inputs["dense_w"], np.float32),
        np.asarray(inputs["dense_b"], np.float32),
    )
    wmap = {k: v.astype(np.float32 if k == "bhm" else np.float16)
            for k, v in ws.items()}

    db = float(np.asarray(inputs["dense_b"], np.float64)[0])

    def one(sl):
        # feat rows [T, 70, 1024] then DoubleRow pack to [T, 35, 2048]
        fx = np.zeros((n_steps, 70, NSG * CK), np.float32)
        dv = dfeat[sl, :n_steps]                     # [BS, T, F]
        dv = dv.reshape(NSG, SGC, CK, n_steps, F)
        fx[:, 0:64, :] = (dv.transpose(3, 1, 4, 0, 2)   # [T,SGC,F,NSG,CK]
                          .reshape(n_steps, 64, NSG * CK))
        fx[:, 64, :] = 1.0
        yv = y0[sl, 0].reshape(NSG, SGC, CK)             # [g, ci, b]
        fx[0, 65:69, :] = (yv.transpose(1, 0, 2)
                           .reshape(4, NSG * CK) - db)
        m = {
            "feat": np.ascontiguousarray(fx).astype(np.float16),
            "h0": np.ascontiguousarray(h0[sl].T).astype(np.float16),
        }
        m.update(wmap)
        return m

    in_maps = [one(slice(i * BS, (i + 1) * BS)) for i in range(NCORES)]
    return in_maps, float(np.asarray(inputs["dense_b"], np.float64)[0])


def run(inputs, trace=False, n_steps=T, **spmd_kwargs):
    """Run on the 8 NeuronCores; returns (out [B,T,1] fp32, results)."""
    from concourse.bass_utils import run_bass_kernel_spmd

    key = n_steps
    if key not in _CACHE:
        _CACHE[key] = _build_module(n_steps)
    nc = _CACHE[key]
    in_maps, db = _host_prep(inputs, n_steps)
    res = run_bass_kernel_spmd(nc, in_maps, list(range(NCORES)),
                               trace=trace, **spmd_kwargs)
    outs = np.concatenate([np.asarray(r["out"]) for r in res.results], axis=1)
    full = (outs.T[:, :, None] + np.float32(db)).astype(np.float32)
    return full, res


def kernel(**inputs) -> np.ndarray:
    out, _ = run(inputs, trace=False)
    return out


# revision 20
# speedup vs baseline: 1.2491x; 1.2491x over previous
"""GRU decoder (nn_Decoder2) Trainium2 Bass kernel, v2.

Per core (pure batch data-parallel over 8 cores): 4096 rows, 8 chunks of 512,
2 supergroups (SG) of 4 chunks, partition-stacked [128, 512] so elementwise
runs 128 lanes wide.  HW: ~289us (baseline 572us), rel err 6.2e-4.

Structure (vs v1's 4 narrow matmuls per gate-bank per SG):
  - Each gate bank [128, 512] (4 chunks stacked at partition 32*ci) is
    computed by TWO K-packed matmuls:
      feat-mm: lhsT [65, 128] block-diag feat weights (rows 16ci+f), row 64
               = ones/bias row; rhs = packed feat [65, 512]; start=True.
      h-mm:    lhsT [128, 128] block-diag recurrent weights; rhs = the
               stacked state tile h_s [128, 512] read DIRECTLY; no
               SBUF->SBUF state-scatter DMAs exist at all.
    mh has no x-side term (bias applied inside the t2 scalar_tensor_tensor);
    7 gate matmuls per SG per step vs 16 in v1.
  - t3 = t2 + xh is done ON THE PE: an identity-matmul accumulates t2 into
    the xh psum bank (closing its group); tanh then reads psum directly.
    This keeps DVE to 4 ops/SG: stt t2, d = h-hh, e = z*d, h' = hh+e
    (the last three all-fp16 SBUF -> DVE 2x mode, ~420ns each).
  - Sigmoid split R-then-Z [128, 512] each (r reaches the t2 chain ~0.5us
    earlier than a fused [128, 1024] sigmoid).
  - h_s is 5x ring-buffered: blend(t) writes buf[t%5], h-mms(t) read
    buf[(t+4)%5], so the dense head can read states up to 4 steps late.
  - Dense head batched per 4 steps: 8 mms (wd4 block-diag, one per step x
    SG) park at partition offsets 32*tau of the freed Z psum region, one
    DVE tensor_copy evacuates all 4 parks, 4 out-DMAs per SG per block.
  - feat prefetched per SG in 4-step blocks [70, 2048] fp16 (rows 65:69 =
    per-chunk (y0 - dense_b) at t=0, zeros later), multi-buffered.
  - Emission order IS the dependency order for the Tile framework: feats
    for step t+1 are emitted after step t's elementwise so the start=True
    bank wipes are WAR-ordered behind this step's psum reads.
Dense(1) -> next-input dependency folded into the recurrent weights
host-side (rk + dw x k0, bias += db*k0), so the recurrence never waits on
the dense output; y0 enters via feat rows 65:69 at t=0 (the y0-db trick
makes one weight set serve all t); host adds dense_b at the end.  Weights
are prepared in float64 and quantized to fp16 (PSUM accumulates fp32).
"""
import numpy as np

B, T, F, H = 32768, 48, 16, 32
NCORES = 8
BS = B // NCORES            # 4096 batch per core
CK = 512                    # chunk batch size
NSG = 2                     # supergroups
SGC = 4                     # chunks per supergroup

_CACHE = {}


def _prep_weights(kernel, recurrent_kernel, bias_x, bias_h, dense_w, dense_b):
    """Build v2 weight tiles in float64, return fp32 dict."""
    kd = kernel.astype(np.float64)
    rkd = recurrent_kernel.astype(np.float64)
    bxd = bias_x.astype(np.float64)
    bhd = bias_h.astype(np.float64)
    dwd = dense_w.astype(np.float64)[:, 0]          # [32]
    dbd = float(dense_b.astype(np.float64)[0])

    k0 = kd[0]                                      # [96]
    kf = kd[1:]                                     # [16, 96]
    dwk0 = np.outer(dwd, k0)                        # [32, 96]

    out = {}
    blocks = {"z": slice(0, 32), "r": slice(32, 64), "x": slice(64, 96)}
    bias1 = {"z": bxd[0:32] + bhd[0:32], "r": bxd[32:64] + bhd[32:64],
             "x": bxd[64:96]}
    for g, blk in blocks.items():
        # Unified feat weights, all t (t=0 handled by y0-db rhs rows):
        # row 0:64 = block-diag kf, 64 = bias1 + db*k0, 65:69 = k0 per
        # chunk, 69 = pad. DoubleRow-packed [35, 2, 128] -> [35, 256].
        wfr = np.zeros((70, 128), np.float64)
        whh = np.zeros((128, 128), np.float64)
        for c in range(4):
            cols = slice(32 * c, 32 * c + 32)
            wfr[16 * c:16 * c + 16, cols] = kf[:, blk]
            wfr[64, cols] = bias1[g] + dbd * k0[blk]
            wfr[65 + c, cols] = k0[blk]
            rows = slice(32 * c, 32 * c + 32)
            if g == "x":
                whh[rows, cols] = dwk0[:, blk]
            else:
                whh[rows, cols] = rkd[:, blk] + dwk0[:, blk]
        out[f"wf_{g}"] = wfr                    # [70, 128] unified, all t
        out[f"whh_{g}"] = whh
        if g != "x":
            wh0 = np.zeros((128, 128), np.float64)
            for c in range(4):
                wh0[32 * c:32 * c + 32, 32 * c:32 * c + 32] = rkd[:, blk]
            out[f"wh0_{g}"] = wh0
    whm = np.zeros((128, 128), np.float64)          # mh: h-only, all t
    for c in range(4):
        whm[32 * c:32 * c + 32, 32 * c:32 * c + 32] = rkd[:, 64:96]
    out["whh_m"] = whm
    wd4 = np.zeros((128, 4), np.float64)
    for c in range(4):
        wd4[32 * c:32 * c + 32, c] = dwd
    out["wd4"] = wd4
    out["bhm"] = np.tile(bhd[64:96], 4).reshape(128, 1)
    out["ident"] = np.eye(128)
    return {k: np.ascontiguousarray(v.astype(np.float32)) for k, v in out.items()}


def _build_module(n_steps=T):
    import concourse.bacc as bacc
    import concourse.mybir as mybir
    import concourse.tile as tile
    from contextlib import ExitStack

    f32 = mybir.dt.float32
    f16 = mybir.dt.float16
    AF = mybir.ActivationFunctionType
    ALU = mybir.AluOpType

    nc = bacc.Bacc("TRN2")
    # feat_packed [T, 70, 1024] fp16: rows 0:64 = chunk-stacked feats,
    # row 64 = ones, rows 65:69 = per-chunk (y0-db) at t=0 / zeros later.
    feat = nc.dram_tensor("feat", [n_steps, 70, NSG * CK], f16,
                          kind="ExternalInput")
    h0 = nc.dram_tensor("h0", [H, BS], f16, kind="ExternalInput")
    wnames_f = ["wf_z", "wf_r", "wf_x"]
    wnames_h = ["whh_z", "whh_r", "whh_x", "whh_m"]
    wnames_h0 = ["wh0_z", "wh0_r", "whh_m"]
    dram_w = {}
    for n in wnames_f:
        dram_w[n] = nc.dram_tensor(n, [70, 128], f16, kind="ExternalInput")
    for n in set(wnames_h + wnames_h0):
        dram_w[n] = nc.dram_tensor(n, [128, 128], f16, kind="ExternalInput")
    dram_w["wd4"] = nc.dram_tensor("wd4", [128, 4], f16, kind="ExternalInput")
    dram_w["ident"] = nc.dram_tensor("ident", [128, 128], f16,
                                     kind="ExternalInput")
    dram_w["bhm"] = nc.dram_tensor("bhm", [128, 1], f32, kind="ExternalInput")
    out = nc.dram_tensor("out", [n_steps, BS], f32, kind="ExternalOutput")

    NBLK = (n_steps + 3) // 4

    with tile.TileContext(nc) as tc, ExitStack() as ctx:
        wpool = ctx.enter_context(tc.tile_pool(name="weights", bufs=1))
        xpool = ctx.enter_context(tc.tile_pool(name="featp", bufs=3))
        hpool = ctx.enter_context(tc.tile_pool(name="hs", bufs=1))
        ew = ctx.enter_context(tc.tile_pool(name="ew", bufs=3))
        dpool = ctx.enter_context(tc.tile_pool(name="dsb", bufs=2))
        ppool = ctx.enter_context(tc.tile_pool(name="psum", bufs=1,
                                               space="PSUM"))

        ws = {}
        for n, d in dram_w.items():
            dt = f32 if n == "bhm" else f16
            t_ = wpool.tile(list(d.shape), dt, tag=n)
            nc.sync.dma_start(t_[:, :], d[:, :])
            ws[n] = t_

        # 5x-buffered stacked state tiles per SG [128, 512] fp16:
        # blend(t) writes buf[t%5]; h-mms(t) read buf[(t+4)%5]; the dense
        # head reads buf[tau%5] up to 5 steps later. h0 preloaded to buf 4.
        h_s = [[hpool.tile([128, CK], f16, tag=f"hs{g}_{p}", name=f"hs{g}_{p}")
                for p in range(5)] for g in range(NSG)]
        for g in range(NSG):
            for ci in range(SGC):
                c = g * SGC + ci
                nc.sync.dma_start(h_s[g][4][32 * ci:32 * ci + 32, :],
                                  h0[:, c * CK:(c + 1) * CK])

        # Feat block tiles [70, 2048] per SG (block b = steps 4b..4b+3).
        feat_t = [[None] * NBLK for _ in range(NSG)]

        def load_block(b):
            t0 = 4 * b
            ns = min(t0 + 4, n_steps) - t0
            for g in range(NSG):
                t_ = xpool.tile([70, 4 * CK], f16, tag=f"feat{g}",
                                name=f"feat{g}_b{b}")
                nc.sync.dma_start(
                    t_[0:70, 0:ns * CK].rearrange("r (s b) -> r s b", s=ns),
                    feat[t0:t0 + ns, :, g * CK:(g + 1) * CK].rearrange(
                        "s r b -> r s b"))
                feat_t[g][b] = t_

        load_block(0)
        if NBLK > 1:
            load_block(1)

        zr_ps = [ppool.tile([128, 2 * CK], f32, tag=f"zr{g}", name=f"zr{g}")
                 for g in range(NSG)]
        xm_ps = [ppool.tile([128, 2 * CK], f32, tag=f"xm{g}", name=f"xm{g}")
                 for g in range(NSG)]

        def gate_bank(g, suf):
            if suf == "_r":
                return zr_ps[g][:, CK:2 * CK]
            if suf == "_z":
                return zr_ps[g][:, 0:CK]
            if suf == "_m":
                return xm_ps[g][:, CK:2 * CK]
            return xm_ps[g][:, 0:CK]       # _x

        def emit_feat(tt):
            bb, ss = tt // 4, tt % 4
            kk = 69 if tt == 0 else 65
            for suf in ["_r", "_z", "_x"]:
                wn = [n for n in wnames_f if n.endswith(suf)][0]
                for g in range(NSG):
                    rhs = feat_t[g][bb][0:kk, ss * CK:(ss + 1) * CK]
                    nc.tensor.matmul(gate_bank(g, suf),
                                     lhsT=ws[wn][0:kk, :], rhs=rhs,
                                     start=True, stop=False,
                                     tile_position=(0, 0))

        def emit_dense_block(t0, nt):
            """Dense mms for steps t0..t0+nt-1, parked at partition offset
            32*tau of the Z psum region, one ACT evac, per-step out-DMAs.
            Emitted during step t0+4 (or after the loop)."""
            for g in range(NSG):
                for tau in range(nt):
                    p0 = 32 * tau
                    nc.tensor.matmul(zr_ps[g][p0:p0 + 4, 0:CK],
                                     lhsT=ws["wd4"][:, :],
                                     rhs=h_s[g][(t0 + tau) % 5][:, :],
                                     start=True, stop=True,
                                     tile_position=(0, p0))
            for g in range(NSG):
                npp = 32 * (nt - 1) + 4
                dsb = dpool.tile([100, CK], f32, tag=f"dsb{g}",
                                 name=f"dsb{g}_{t0}")
                nc.vector.tensor_copy(dsb[0:npp, :], zr_ps[g][0:npp, 0:CK])
                gb = g * SGC * CK
                for tau in range(nt):
                    nc.sync.dma_start(
                        out[t0 + tau, gb:gb + 4 * CK].rearrange(
                            "(c b) -> c b", c=4),
                        dsb[32 * tau:32 * tau + 4, :])

        for t in range(n_steps):
            blk, slot = t // 4, t % 4
            wh = wnames_h0 if t == 0 else wnames_h  # feat weights unified
            hb_in = [h_s[g][(t + 4) % 5] for g in range(NSG)]

            if t == 0:
                emit_feat(0)

            # -- PE: h-mms, gate-paired across SGs; r first ------------
            for suf in ["_r", "_z", "_m", "_x"]:
                wn = [n for n in wh if n.endswith(suf)]
                if not wn:
                    continue
                for g in range(NSG):
                    # xh bank is closed by the t2-inject mm, not here
                    nc.tensor.matmul(gate_bank(g, suf), lhsT=ws[wn[0]][:, :],
                                     rhs=hb_in[g][:, :],
                                     start=(suf == "_m"),
                                     stop=(suf != "_x"),
                                     tile_position=(0, 0))

            # -- elementwise per SG, op-major emission ----------------
            ewt = {}
            for g in range(NSG):
                ewt[g] = (
                    ew.tile([128, 2 * CK], f16, tag=f"zrs{g}",
                            name=f"zrs{g}_{t}"),
                    ew.tile([128, CK], f16, tag=f"t2s{g}", name=f"t2s{g}_{t}"),
                    None,
                    ew.tile([128, CK], f16, tag=f"hhs{g}", name=f"hhs{g}_{t}"),
                    ew.tile([128, CK], f16, tag=f"ds{g}", name=f"ds{g}_{t}"),
                    ew.tile([128, CK], f16, tag=f"es{g}", name=f"es{g}_{t}"),
                )
            for g in range(NSG):
                nc.scalar.activation(ewt[g][0][:, CK:2 * CK],
                                     zr_ps[g][:, CK:2 * CK], AF.Sigmoid)
            for g in range(NSG):
                nc.vector.scalar_tensor_tensor(
                    ewt[g][1][:, :], xm_ps[g][:, CK:2 * CK], ws["bhm"][:, 0:1],
                    ewt[g][0][:, CK:2 * CK], ALU.add, ALU.mult)
            for g in range(NSG):
                nc.scalar.activation(ewt[g][0][:, 0:CK],
                                     zr_ps[g][:, 0:CK], AF.Sigmoid)
            for g in range(NSG):
                # t3: accumulate t2 into the xh psum bank on PE (identity
                # matmul, closes the bank group); tanh then reads psum.
                nc.tensor.matmul(xm_ps[g][:, 0:CK], lhsT=ws["ident"][:, :],
                                 rhs=ewt[g][1][:, :], start=False, stop=True,
                                 tile_position=(0, 0))
            for g in range(NSG):
                nc.scalar.activation(ewt[g][3][:, :], xm_ps[g][:, 0:CK],
                                     AF.Tanh)
            for g in range(NSG):
                nc.vector.tensor_sub(ewt[g][4][:, :], hb_in[g][:, :],
                                     ewt[g][3][:, :])
            for g in range(NSG):
                nc.vector.tensor_mul(ewt[g][5][:, :], ewt[g][0][:, 0:CK],
                                     ewt[g][4][:, :])
            for g in range(NSG):
                nc.vector.tensor_add(h_s[g][t % 5][:, :], ewt[g][3][:, :],
                                     ewt[g][5][:, :])

            # -- PE: dense block for previous 4 steps (after sig(t) so the
            #    parked writes are WAR-ordered behind the Z-region read) --
            if t > 0 and t % 4 == 0:
                emit_dense_block(t - 4, 4)

            # -- PE: feat-mms for next step (after sig(t)/t3(t): their
            #    start=True wipes must be ordered behind this step's reads)
            if t + 1 < n_steps:
                emit_feat(t + 1)

            # -- prefetch feat block -----------------------------------
            if slot == 3 and blk + 2 < NBLK:
                load_block(blk + 2)

        last0 = (n_steps - 1) // 4 * 4
        emit_dense_block(last0, n_steps - last0)
    nc.compile()
    return nc


def _host_prep(inputs, n_steps=T):
    """Shard + pack inputs host-side. Returns (in_maps, dense_b)."""
    dfeat = np.asarray(inputs["decoder_feature"], np.float32)
    y0 = np.asarray(inputs["decoder_init_input"], np.float32)
    h0 = np.asarray(inputs["init_state"], np.float32)
    ws = _prep_weights(
        np.asarray(inputs["kernel"], np.float32),
        np.asarray(inputs["recurrent_kernel"], np.float32),
        np.asarray(inputs["bias_x"], np.float32),
        np.asarray(inputs["bias_h"], np.float32),
        np.asarray(inputs["dense_w"], np.float32),
        np.asarray(inputs["dense_b"], np.float32),
    )
    wmap = {k: v.astype(np.float32 if k == "bhm" else np.float16)
            for k, v in ws.items()}

    db = float(np.asarray(inputs["dense_b"], np.float64)[0])

    def one(sl):
        # feat rows [T, 70, 1024] then DoubleRow pack to [T, 35, 2048]
        fx = np.zeros((n_steps, 70, NSG * CK), np.float32)
        dv = dfeat[sl, :n_steps]                     # [BS, T, F]
        dv = dv.reshape(NSG, SGC, CK, n_steps, F)
        fx[:, 0:64, :] = (dv.transpose(3, 1, 4, 0, 2)   # [T,SGC,F,NSG,CK]
                          .reshape(n_steps, 64, NSG * CK))
        fx[:, 64, :] = 1.0
        yv = y0[sl, 0].reshape(NSG, SGC, CK)             # [g, ci, b]
        fx[0, 65:69, :] = (yv.transpose(1, 0, 2)
                           .reshape(4, NSG * CK) - db)
        m = {
            "feat": np.ascontiguousarray(fx).astype(np.float16),
            "h0": np.ascontiguousarray(h0[sl].T).astype(np.float16),
        }
        m.update(wmap)
        return m

    in_maps = [one(slice(i * BS, (i + 1) * BS)) for i in range(NCORES)]
    return in_maps, float(np.asarray(inputs["dense_b"], np.float64)[0])


def run(inputs, trace=False, n_steps=T, **spmd_kwargs):
    """Run on the 8 NeuronCores; returns (out [B,T,1] fp32, results)."""
    from concourse.bass_utils import run_bass_kernel_spmd

    key = n_steps
    if key not in _CACHE:
        _CACHE[key] = _build_module(n_steps)
    nc = _CACHE[key]
    in_maps, db = _host_prep(inputs, n_steps)
    res = run_bass_kernel_spmd(nc, in_maps, list(range(NCORES)),
                               trace=trace, **spmd_kwargs)
    outs = np.concatenate([np.asarray(r["out"]) for r in res.results], axis=1)
    full = (outs.T[:, :, None] + np.float32(db)).astype(np.float32)
    return full, res


def kernel(**inputs) -> np.ndarray:
    out, _ = run(inputs, trace=False)
    return out
